# revision 11
# baseline (speedup 1.0000x reference)
"""MoE (top-2 of 8 experts, SwiGLU) kernel for 8 Trainium2 NeuronCores.

Strategy (expert-parallel, per the sharding hint):
  - Host computes the token-choice router (tiny: [8192,1024]@[1024,8]) and
    the two scalar aux losses, plus the dispatch metadata (per-expert token
    index lists).  Each of the 8 cores owns one expert.
  - Each core receives its expert's gathered+transposed tokens
    xgT [H, C] (C = padded capacity), its expert weights, and the per-slot
    routing weight.  On device it computes
        out = (silu(x@wg) * (x@wu)) @ wd, scaled by the routing weight,
    entirely in float32r matmuls (TF32-class precision, bf16-class speed).
  - Host combines: final[t] = contrib[expert0(t)] + contrib[expert1(t)],
    clipped to +-10.

Device blocking (per core):
  C tokens split into blocks of CB=1152.  Within a block, loop over
  I-chunks of IC=512: compute G^T/U^T [IC, CB] via PE (weights stationary,
  tokens moving), fuse silu*mul into aT chunk, then immediately run the
  down-projection for the chunk and accumulate OUT [CB, H] in SBUF.
  Weights stream HBM->SBUF once per block (2 blocks => 100MB; hidden
  under ~800us of PE work).
"""
import numpy as np
from contextlib import ExitStack

import concourse.bacc as bacc
import concourse.tile as tile
import concourse.mybir as mybir
from concourse import bass_utils

TOP_K = 2
AUX_COEF = 0.001
B, S, H, I, E = 4, 2048, 1024, 4096, 8
T = B * S
N_CORES = 8

CB = 768         # tokens per block on device
IC = 512         # I-chunk size
CSUB = 384       # moving-dim subtile for gate/up matmuls
NT = 512         # moving-dim subtile for down-proj matmuls

F32 = mybir.dt.float32
F32R = mybir.dt.float32r

_nc_cache = {}


def _build_nc(C):
    assert C % CB == 0
    n_blocks = C // CB
    n_chunks = I // IC          # 8
    n_it = IC // 128            # 4 i-tiles per chunk
    n_cs = CB // CSUB           # 3 c-subtiles per block
    n_m = CB // 128             # 9 m-tiles per block
    n_n = H // NT               # 2 n-tiles
    n_kc = IC // 128            # 4 k-tiles per chunk (down-proj)
    n_kh = H // 128             # 8 k-tiles (gate/up contraction over H)

    nc = bacc.Bacc("TRN2", target_bir_lowering=False, debug=False)
    xgt = nc.dram_tensor("xgt", [H, C], F32R, kind="ExternalInput")
    wg = nc.dram_tensor("wg", [H, I], F32R, kind="ExternalInput")
    wu = nc.dram_tensor("wu", [H, I], F32R, kind="ExternalInput")
    wd = nc.dram_tensor("wd", [I, H], F32R, kind="ExternalInput")
    scale = nc.dram_tensor("scale", [128, C // 128], F32, kind="ExternalInput")
    outc = nc.dram_tensor("outc", [C, H], F32, kind="ExternalOutput")

    xgt_r = xgt.rearrange("(ko p) c -> p ko c", p=128)     # [128, 8, C]
    wg_r = wg.rearrange("(ko p) i -> p ko i", p=128)       # [128, 8, I]
    wu_r = wu.rearrange("(ko p) i -> p ko i", p=128)
    wd_r = wd.rearrange("(ko p) h -> p ko h", p=128)       # [128, 32, H]
    outc_r = outc.rearrange("(mt p) h -> p mt h", p=128)   # [128, C/128, H]

    IH = IC // 2  # half-chunk granularity for gate/up weight DMAs

    with tile.TileContext(nc) as tc, ExitStack() as ctx:
        xg_pool = ctx.enter_context(tc.tile_pool(name="xg", bufs=2 * n_cs))
        wg_pool = ctx.enter_context(tc.tile_pool(name="wg", bufs=4))
        wu_pool = ctx.enter_context(tc.tile_pool(name="wu", bufs=4))
        wd_pool = ctx.enter_context(tc.tile_pool(name="wdp", bufs=2))
        a_pool = ctx.enter_context(tc.tile_pool(name="at", bufs=1))
        silu_pool = ctx.enter_context(tc.tile_pool(name="silu", bufs=3))
        out_pool = ctx.enter_context(tc.tile_pool(name="outsb", bufs=1))
        sc_pool = ctx.enter_context(tc.tile_pool(name="sc", bufs=1))
        psg_pool = ctx.enter_context(tc.tile_pool(name="psg", bufs=2, space="PSUM"))
        psu_pool = ctx.enter_context(tc.tile_pool(name="psu", bufs=2, space="PSUM"))
        pso_pool = ctx.enter_context(tc.tile_pool(name="pso", bufs=4, space="PSUM"))

        sc_t = sc_pool.tile([128, C // 128], F32)
        nc.sync.dma_start(sc_t[:], scale[:])

        # PE warmup: dummy matmuls with no DMA dependency keep the PE busy
        # while the first real loads land, and un-throttle the HAM clock
        # gate (cold 1.2GHz -> warm 2.4GHz) before real work starts.
        wu_sb = sc_pool.tile([128, 640], F32R, tag="warm")
        nc.vector.memset(wu_sb.bitcast(F32)[:], 0.0)
        for g in range(3):
            wu_ps = pso_pool.tile([128, NT], F32, tag="ps_o")
            for k in range(8):
                nc.tensor.matmul(
                    wu_ps[:], wu_sb[:, 0:128], wu_sb[:, 128:128 + NT],
                    start=(k == 0), stop=(k == 7),
                )

        def load_gu(i0, h):
            wg_t = wg_pool.tile([128, n_kh, IH], F32R, tag="wg")
            nc.sync.dma_start(
                wg_t[:], wg_r[:, :, i0 + h * IH:i0 + (h + 1) * IH])
            wu_t = wu_pool.tile([128, n_kh, IH], F32R, tag="wu")
            nc.sync.dma_start(
                wu_t[:], wu_r[:, :, i0 + h * IH:i0 + (h + 1) * IH])
            return wg_t, wu_t

        def load_wd(ci):
            wd_t = wd_pool.tile([128, n_kc, H], F32R)
            nc.sync.dma_start(wd_t[:], wd_r[:, ci * n_kc:(ci + 1) * n_kc, :])
            return wd_t

        for b in range(n_blocks):
            c0 = b * CB
            # DMA issue order matters at block 0: first c-subtile of xgT and
            # the first gate/up half-chunk land first so PE starts ASAP.
            xg_ts = []
            xg_t = xg_pool.tile([128, n_kh, CSUB], F32R, tag="xg")
            nc.sync.dma_start(xg_t[:], xgt_r[:, :, c0:c0 + CSUB])
            xg_ts.append(xg_t)
            g0, u0 = load_gu(0, 0)
            for cs in range(1, n_cs):
                s0 = c0 + cs * CSUB
                xg_t = xg_pool.tile([128, n_kh, CSUB], F32R, tag="xg")
                nc.sync.dma_start(xg_t[:], xgt_r[:, :, s0:s0 + CSUB])
                xg_ts.append(xg_t)
            g1, u1 = load_gu(0, 1)
            wd0 = load_wd(0)
            out_sb = out_pool.tile([128, n_m, H], F32)

            for ci in range(n_chunks):
                i0 = ci * IC
                if ci == 0:
                    wg_ts, wu_ts, wd_t = [g0, g1], [u0, u1], wd0
                else:
                    wg_ts, wu_ts = [], []
                    for h in range(2):
                        wg_t, wu_t = load_gu(i0, h)
                        wg_ts.append(wg_t)
                        wu_ts.append(wu_t)
                    wd_t = load_wd(ci)

                a_t = a_pool.tile([128, n_kc, CB], F32R)

                for it in range(n_it):
                    wg_h = wg_ts[it // (n_it // 2)]
                    wu_h = wu_ts[it // (n_it // 2)]
                    ih = (it % (n_it // 2)) * 128
                    for cs in range(n_cs):
                        s0 = cs * CSUB
                        ps_g = psg_pool.tile([128, CSUB], F32)
                        for k in range(n_kh):
                            nc.tensor.matmul(
                                ps_g[:],
                                wg_h[:, k, ih:ih + 128],
                                xg_ts[cs][:, k, :],
                                start=(k == 0), stop=(k == n_kh - 1),
                            )
                        ps_u = psu_pool.tile([128, CSUB], F32)
                        for k in range(n_kh):
                            nc.tensor.matmul(
                                ps_u[:],
                                wu_h[:, k, ih:ih + 128],
                                xg_ts[cs][:, k, :],
                                start=(k == 0), stop=(k == n_kh - 1),
                            )
                        sl_t = silu_pool.tile([128, CSUB], F32)
                        nc.scalar.activation(
                            sl_t[:], ps_g[:], mybir.ActivationFunctionType.Silu
                        )
                        nc.vector.tensor_mul(
                            a_t[:, it, s0:s0 + CSUB], sl_t[:], ps_u[:]
                        )

                for m in range(n_m):
                    for n in range(n_n):
                        ps_o = pso_pool.tile([128, NT], F32)
                        for k in range(n_kc):
                            nc.tensor.matmul(
                                ps_o[:],
                                a_t[:, k, m * 128:(m + 1) * 128],
                                wd_t[:, k, n * NT:(n + 1) * NT],
                                start=(k == 0), stop=(k == n_kc - 1),
                            )
                        dst = out_sb[:, m, n * NT:(n + 1) * NT]
                        sc_ap = sc_t[:, b * n_m + m:b * n_m + m + 1]
                        if ci == 0:
                            nc.vector.tensor_scalar_mul(dst, ps_o[:], sc_ap)
                        else:
                            nc.vector.scalar_tensor_tensor(
                                dst, ps_o[:], sc_ap, dst,
                                op0=mybir.AluOpType.mult,
                                op1=mybir.AluOpType.add,
                            )

            for m in range(n_m):
                nc.sync.dma_start(
                    outc_r[:, b * n_m + m:b * n_m + m + 1, :],
                    out_sb[:, m:m + 1, :],
                )

    nc.compile()
    return nc


def get_nc(C):
    if C not in _nc_cache:
        _nc_cache[C] = _build_nc(C)
    return _nc_cache[C]


def _route(hidden_states, router_w):
    """Host router: returns per-core inputs metadata + losses (fp32)."""
    x = np.clip(np.asarray(hidden_states, np.float32), -10.0, 10.0)
    xf = x.reshape(T, H)
    logits = xf @ np.asarray(router_w, np.float32)           # [T, E]
    mx = logits.max(axis=-1, keepdims=True)
    ex = np.exp(logits - mx)
    sex = ex.sum(axis=-1, keepdims=True)
    probs = ex / sex
    lse = (mx + np.log(sex)).ravel()
    z_loss = np.float32(np.mean(lse ** 2))

    i1 = probs.argmax(axis=-1)
    p1 = probs[np.arange(T), i1]
    pm = probs.copy()
    pm[np.arange(T), i1] = -np.inf
    i2 = pm.argmax(axis=-1)
    p2 = probs[np.arange(T), i2]
    denom = p1 + p2
    w1 = p1 / denom
    w2 = p2 / denom

    counts = np.zeros(E, np.int64)
    np.add.at(counts, i1, 1)
    np.add.at(counts, i2, 1)
    frac = counts.astype(np.float32) / T / TOP_K
    lb_loss = np.float32(AUX_COEF * E * np.sum(frac * probs.mean(axis=0)))
    return xf, probs, i1, i2, w1, w2, counts, lb_loss, z_loss


def build_src(i1, i2, C):
    """Host combine indices: token t's two contributions live at rows
    src0[t], src1[t] of the concatenated [E*C, H] device output."""
    src0 = np.empty(T, np.int64)
    src1 = np.empty(T, np.int64)
    for e in range(E):
        idx = np.flatnonzero((i1 == e) | (i2 == e))
        first = i1[idx] == e
        rank = np.arange(len(idx))
        src0[idx[first]] = e * C + rank[first]
        src1[idx[~first]] = e * C + rank[~first]
    return src0, src1


def build_in_maps(inputs, xf, i1, i2, w1, w2, C):
    w_gate = np.ascontiguousarray(np.asarray(inputs["w_gate"], np.float32))
    w_up = np.ascontiguousarray(np.asarray(inputs["w_up"], np.float32))
    w_down = np.ascontiguousarray(np.asarray(inputs["w_down"], np.float32))
    in_maps = []
    for e in range(E):
        idx = np.flatnonzero((i1 == e) | (i2 == e))
        ne = len(idx)
        first = i1[idx] == e
        xg = np.zeros((C, H), np.float32)
        xg[:ne] = xf[idx]
        sc = np.zeros(C, np.float32)
        sc[:ne] = np.where(first, w1[idx], w2[idx]).astype(np.float32)
        in_maps.append({
            "xgt": np.ascontiguousarray(xg.T),
            "wg": w_gate[e],
            "wu": w_up[e],
            "wd": w_down[e],
            "scale": np.ascontiguousarray(sc.reshape(C // 128, 128).T),
        })
    return in_maps


def kernel(**inputs):
    hidden_states = np.asarray(inputs["hidden_states"], np.float32)
    router_w = np.asarray(inputs["router_w"], np.float32)

    xf, probs, i1, i2, w1, w2, counts, lb_loss, z_loss = _route(
        hidden_states, router_w
    )

    C = max(CB, int(-(-counts.max() // CB)) * CB)
    nc = get_nc(C)
    in_maps = build_in_maps(inputs, xf, i1, i2, w1, w2, C)
    src0, src1 = build_src(i1, i2, C)

    res = bass_utils.run_bass_kernel_spmd(
        nc, in_maps, core_ids=list(range(N_CORES))
    )
    outc_all = np.concatenate([res.results[c]["outc"] for c in range(N_CORES)],
                              axis=0)
    out = outc_all[src0] + outc_all[src1]
    out = np.clip(out, -10.0, 10.0).reshape(B, S, H)
    return out, lb_loss, z_loss


# revision 36
# speedup vs baseline: 1.0591x; 1.0591x over previous
"""MoE (top-2 of 8 experts, SwiGLU) kernel for 8 Trainium2 NeuronCores.

Strategy (expert-parallel, per the sharding hint):
  - Host computes the token-choice router (tiny: [8192,1024]@[1024,8]) and
    the two scalar aux losses, plus the dispatch metadata (per-expert token
    index lists).  Each of the 8 cores owns one expert.
  - Each core receives its expert's gathered+transposed tokens
    xgT [H, C] (C = padded capacity), its expert weights, and the per-slot
    routing weight.  On device it computes
        out = (silu(x@wg) * (x@wu)) @ wd, scaled by the routing weight,
    entirely in float32r matmuls (TF32-class precision, bf16-class speed).
  - Host combines: final[t] = contrib[expert0(t)] + contrib[expert1(t)],
    clipped to +-10.

Device blocking (per core):
  C tokens split into blocks of CB=1152.  Within a block, loop over
  I-chunks of IC=512: compute G^T/U^T [IC, CB] via PE (weights stationary,
  tokens moving), fuse silu*mul into aT chunk, then immediately run the
  down-projection for the chunk and accumulate OUT [CB, H] in SBUF.
  Weights stream HBM->SBUF once per block (2 blocks => 100MB; hidden
  under ~800us of PE work).
"""
import numpy as np
from contextlib import ExitStack

import concourse.bacc as bacc
import concourse.tile as tile
import concourse.mybir as mybir
from concourse import bass_utils

TOP_K = 2
AUX_COEF = 0.001
B, S, H, I, E = 4, 2048, 1024, 4096, 8
T = B * S
N_CORES = 8

CB = 768         # tokens per block on device
IC = 512         # I-chunk size
CSUB = 384       # moving-dim subtile for gate/up matmuls
NT = 512         # moving-dim subtile for down-proj matmuls

F32 = mybir.dt.float32
F32R = mybir.dt.float32r

_nc_cache = {}


def normalize_cx(cx):
    """Exact token capacity, adjusted so the final block decomposes into
    c-subtile widths all >= 256 (f32r matmuls drop to 1/4 rate below 256)
    and 4-aligned (ISA)."""
    cx = max(int(cx), 256)
    cx = -(-cx // 8) * 8
    r = cx % CB
    if r == 0:
        return cx
    if r < 256:
        cx += 256 - r
    elif 384 < r <= 512:
        cx += 520 - r
    return cx


def _block_plan(cx):
    """Partition cx token-columns into blocks (<= CB total each); each block
    is a list of c-subtile widths in [256, 512]."""
    k, r = divmod(cx, CB)
    blocks = [[CSUB, CSUB] for _ in range(k)]
    if r:
        if r <= 384:
            blocks.append([r])
        elif r <= 512:
            blocks.append([r])      # single wide subtile (<= 512)
        else:
            blocks.append([(r + 1) // 2, r // 2])
    return blocks


def _build_nc(Cx):
    blocks = _block_plan(Cx)          # list of width-lists; widths in [256,512]
    Cp = -(-Cx // 128) * 128          # padded row count for I/O arrays
    n_chunks = I // IC                # 8
    n_it = IC // 128                  # 4 i-tiles per chunk
    n_n = H // NT                     # 2 n-tiles
    n_kc = IC // 128                  # 4 k-tiles per chunk (down-proj)
    n_kh = H // 128                   # 8 k-tiles (gate/up contraction over H)
    n_m_max = CB // 128

    nc = bacc.Bacc("TRN2", target_bir_lowering=False, debug=False)
    xgt = nc.dram_tensor("xgt", [H, Cp], F32R, kind="ExternalInput")
    wg = nc.dram_tensor("wg", [H, I], F32R, kind="ExternalInput")
    wu = nc.dram_tensor("wu", [H, I], F32R, kind="ExternalInput")
    wd = nc.dram_tensor("wd", [I, H], F32R, kind="ExternalInput")
    scale = nc.dram_tensor("scale", [128, Cp // 128], F32, kind="ExternalInput")
    outc = nc.dram_tensor("outc", [Cp, H], F32, kind="ExternalOutput")

    xgt_r = xgt.rearrange("(ko p) c -> p ko c", p=128)     # [128, 8, Cp]
    wg_r = wg.rearrange("(ko p) i -> p ko i", p=128)       # [128, 8, I]
    wu_r = wu.rearrange("(ko p) i -> p ko i", p=128)
    wd_r = wd.rearrange("(ko p) h -> p ko h", p=128)       # [128, 32, H]
    outc_r = outc.rearrange("(mt p) h -> p mt h", p=128)   # [128, Cp/128, H]

    IH = IC // 2  # half-chunk granularity for gate/up weight DMAs

    with tile.TileContext(nc) as tc, ExitStack() as ctx:
        xg_pool = ctx.enter_context(tc.tile_pool(name="xg", bufs=4))
        wg_pool = ctx.enter_context(tc.tile_pool(name="wg", bufs=4))
        wu_pool = ctx.enter_context(tc.tile_pool(name="wu", bufs=4))
        wd_pool = ctx.enter_context(tc.tile_pool(name="wdp", bufs=2))
        a_pool = ctx.enter_context(tc.tile_pool(name="at", bufs=1))
        silu_pool = ctx.enter_context(tc.tile_pool(name="silu", bufs=3))
        out_pool = ctx.enter_context(tc.tile_pool(name="outsb", bufs=n_m_max + 3))
        sc_pool = ctx.enter_context(tc.tile_pool(name="sc", bufs=1))
        psg_pool = ctx.enter_context(tc.tile_pool(name="psg", bufs=2, space="PSUM"))
        psu_pool = ctx.enter_context(tc.tile_pool(name="psu", bufs=2, space="PSUM"))
        pso_pool = ctx.enter_context(tc.tile_pool(name="pso", bufs=4, space="PSUM"))

        sc_t = sc_pool.tile([128, Cp // 128], F32)
        nc.sync.dma_start(sc_t[:], scale[:])

        # PE warmup: dummy matmuls with no DMA dependency keep the PE busy
        # while the first real loads land, and un-throttle the HAM clock
        # gate (cold 1.2GHz -> warm 2.4GHz) before real work starts.
        wu_sb = sc_pool.tile([128, 640], F32R, tag="warm")
        nc.vector.memset(wu_sb.bitcast(F32)[:], 0.0)
        for g in range(6):
            wu_ps = pso_pool.tile([128, NT], F32, tag="ps_o")
            for k in range(8):
                nc.tensor.matmul(
                    wu_ps[:], wu_sb[:, 0:128], wu_sb[:, 128:128 + NT],
                    start=(k == 0), stop=(k == 7),
                )

        def load_gu(i0, h):
            wg_t = wg_pool.tile([128, n_kh, IH], F32R, tag="wg")
            nc.sync.dma_start(
                wg_t[:], wg_r[:, :, i0 + h * IH:i0 + (h + 1) * IH])
            wu_t = wu_pool.tile([128, n_kh, IH], F32R, tag="wu")
            nc.sync.dma_start(
                wu_t[:], wu_r[:, :, i0 + h * IH:i0 + (h + 1) * IH])
            return wg_t, wu_t

        def load_wd(ci):
            wd_t = wd_pool.tile([128, n_kc, H], F32R)
            nc.sync.dma_start(wd_t[:], wd_r[:, ci * n_kc:(ci + 1) * n_kc, :])
            return wd_t

        c0 = 0
        for b, widths in enumerate(blocks):
            bt = sum(widths)
            n_cs = len(widths)
            n_m = -(-bt // 128)
            # DMA issue order matters at block 0: first c-subtile of xgT and
            # the first gate/up half-chunk land first so PE starts ASAP.
            xg_ts = []
            xg_t = xg_pool.tile([128, n_kh, widths[0]], F32R, tag="xg")
            nc.sync.dma_start(xg_t[:], xgt_r[:, :, c0:c0 + widths[0]])
            xg_ts.append(xg_t)
            g0, u0 = load_gu(0, 0)
            s0 = widths[0]
            for cs in range(1, n_cs):
                w = widths[cs]
                xg_t = xg_pool.tile([128, n_kh, w], F32R, tag="xg")
                nc.sync.dma_start(xg_t[:], xgt_r[:, :, c0 + s0:c0 + s0 + w])
                xg_ts.append(xg_t)
                s0 += w
            g1, u1 = load_gu(0, 1)
            wd0 = load_wd(0)
            out_ts = []
            for _m in range(n_m):
                out_t = out_pool.tile([128, H], F32, tag="out")
                out_ts.append(out_t)

            for ci in range(n_chunks):
                i0 = ci * IC
                if ci == 0:
                    wg_ts, wu_ts, wd_t = [g0, g1], [u0, u1], wd0
                else:
                    wg_ts, wu_ts = [], []
                    for h in range(2):
                        wg_t, wu_t = load_gu(i0, h)
                        wg_ts.append(wg_t)
                        wu_ts.append(wu_t)
                    wd_t = load_wd(ci)

                a_t = a_pool.tile([128, n_kc, CB], F32R, tag="a_t")

                s0 = 0
                for cs in range(n_cs):
                    w = widths[cs]
                    for it in range(n_it):
                        wg_h = wg_ts[it // (n_it // 2)]
                        wu_h = wu_ts[it // (n_it // 2)]
                        ih = (it % (n_it // 2)) * 128
                        ps_g = psg_pool.tile([128, w], F32, tag="ps_g")
                        for k in range(n_kh):
                            nc.tensor.matmul(
                                ps_g[:],
                                wg_h[:, k, ih:ih + 128],
                                xg_ts[cs][:, k, :],
                                start=(k == 0), stop=(k == n_kh - 1),
                            )
                        ps_u = psu_pool.tile([128, w], F32, tag="ps_u")
                        for k in range(n_kh):
                            nc.tensor.matmul(
                                ps_u[:],
                                wu_h[:, k, ih:ih + 128],
                                xg_ts[cs][:, k, :],
                                start=(k == 0), stop=(k == n_kh - 1),
                            )
                        sl_t = silu_pool.tile([128, w], F32, tag="sl")
                        nc.scalar.activation(
                            sl_t[:], ps_g[:], mybir.ActivationFunctionType.Silu
                        )
                        nc.vector.tensor_mul(
                            a_t[:, it, s0:s0 + w], sl_t[:], ps_u[:]
                        )
                    s0 += w

                for m in range(n_m):
                    mw = min(128, bt - m * 128)
                    for n in range(n_n):
                        ps_o = pso_pool.tile([128, NT], F32, tag="ps_o")
                        for k in range(n_kc):
                            # stationary stays full 128-wide (ISA requires
                            # it); rows >= mw are garbage and never read
                            nc.tensor.matmul(
                                ps_o[:],
                                a_t[:, k, m * 128:m * 128 + 128],
                                wd_t[:, k, n * NT:(n + 1) * NT],
                                start=(k == 0), stop=(k == n_kc - 1),
                            )
                        dst = out_ts[m][0:mw, n * NT:(n + 1) * NT]
                        idx = c0 // 128 + m
                        sc_ap = sc_t[0:mw, idx:idx + 1]
                        if ci == 0:
                            nc.vector.tensor_scalar_mul(dst, ps_o[0:mw, :], sc_ap)
                        else:
                            nc.vector.scalar_tensor_tensor(
                                dst, ps_o[0:mw, :], sc_ap, dst,
                                op0=mybir.AluOpType.mult,
                                op1=mybir.AluOpType.add,
                            )

            for m in range(n_m):
                mw = min(128, bt - m * 128)
                nc.sync.dma_start(
                    outc_r[0:mw, c0 // 128 + m, :], out_ts[m][0:mw, :]
                )
            c0 += bt

    nc.compile()
    return nc


def capacity(counts):
    """Exact (normalized) device token capacity Cx."""
    return normalize_cx(int(counts.max()))


def padded(cx):
    """Row-padded capacity (I/O array sizing)."""
    return -(-cx // 128) * 128


def get_nc(C):
    if C not in _nc_cache:
        _nc_cache[C] = _build_nc(C)
    return _nc_cache[C]


def _route(hidden_states, router_w):
    """Host router: returns per-core inputs metadata + losses (fp32)."""
    x = np.clip(np.asarray(hidden_states, np.float32), -10.0, 10.0)
    xf = x.reshape(T, H)
    logits = xf @ np.asarray(router_w, np.float32)           # [T, E]
    mx = logits.max(axis=-1, keepdims=True)
    ex = np.exp(logits - mx)
    sex = ex.sum(axis=-1, keepdims=True)
    probs = ex / sex
    lse = (mx + np.log(sex)).ravel()
    z_loss = np.float32(np.mean(lse ** 2))

    i1 = probs.argmax(axis=-1)
    p1 = probs[np.arange(T), i1]
    pm = probs.copy()
    pm[np.arange(T), i1] = -np.inf
    i2 = pm.argmax(axis=-1)
    p2 = probs[np.arange(T), i2]
    denom = p1 + p2
    w1 = p1 / denom
    w2 = p2 / denom

    counts = np.zeros(E, np.int64)
    np.add.at(counts, i1, 1)
    np.add.at(counts, i2, 1)
    frac = counts.astype(np.float32) / T / TOP_K
    lb_loss = np.float32(AUX_COEF * E * np.sum(frac * probs.mean(axis=0)))
    return xf, probs, i1, i2, w1, w2, counts, lb_loss, z_loss


def build_src(i1, i2, C):
    """Host combine indices: token t's two contributions live at rows
    src0[t], src1[t] of the concatenated [E*C, H] device output."""
    src0 = np.empty(T, np.int64)
    src1 = np.empty(T, np.int64)
    for e in range(E):
        idx = np.flatnonzero((i1 == e) | (i2 == e))
        first = i1[idx] == e
        rank = np.arange(len(idx))
        src0[idx[first]] = e * C + rank[first]
        src1[idx[~first]] = e * C + rank[~first]
    return src0, src1


def build_in_maps(inputs, xf, i1, i2, w1, w2, C):
    w_gate = np.ascontiguousarray(np.asarray(inputs["w_gate"], np.float32))
    w_up = np.ascontiguousarray(np.asarray(inputs["w_up"], np.float32))
    w_down = np.ascontiguousarray(np.asarray(inputs["w_down"], np.float32))
    in_maps = []
    for e in range(E):
        idx = np.flatnonzero((i1 == e) | (i2 == e))
        ne = len(idx)
        first = i1[idx] == e
        xg = np.zeros((C, H), np.float32)
        xg[:ne] = xf[idx]
        sc = np.zeros(C, np.float32)
        sc[:ne] = np.where(first, w1[idx], w2[idx]).astype(np.float32)
        in_maps.append({
            "xgt": np.ascontiguousarray(xg.T),
            "wg": w_gate[e],
            "wu": w_up[e],
            "wd": w_down[e],
            "scale": np.ascontiguousarray(sc.reshape(C // 128, 128).T),
        })
    return in_maps


def kernel(**inputs):
    hidden_states = np.asarray(inputs["hidden_states"], np.float32)
    router_w = np.asarray(inputs["router_w"], np.float32)

    xf, probs, i1, i2, w1, w2, counts, lb_loss, z_loss = _route(
        hidden_states, router_w
    )

    Cx = capacity(counts)
    nc = get_nc(Cx)
    Cp = padded(Cx)
    in_maps = build_in_maps(inputs, xf, i1, i2, w1, w2, Cp)
    src0, src1 = build_src(i1, i2, Cp)

    res = bass_utils.run_bass_kernel_spmd(
        nc, in_maps, core_ids=list(range(N_CORES))
    )
    outc_all = np.concatenate([res.results[c]["outc"] for c in range(N_CORES)],
                              axis=0)
    out = outc_all[src0] + outc_all[src1]
    out = np.clip(out, -10.0, 10.0).reshape(B, S, H)
    return out, lb_loss, z_loss


# revision 42
# speedup vs baseline: 1.0599x; 1.0008x over previous
"""MoE (top-2 of 8 experts, SwiGLU) kernel for 8 Trainium2 NeuronCores.

Strategy (expert-parallel, per the sharding hint):
  - Host computes the token-choice router (tiny: [8192,1024]@[1024,8]) and
    the two scalar aux losses, plus the dispatch metadata (per-expert token
    index lists).  Each of the 8 cores owns one expert.
  - Each core receives its expert's gathered+transposed tokens
    xgT [H, C] (C = padded capacity), its expert weights, and the per-slot
    routing weight.  On device it computes
        out = (silu(x@wg) * (x@wu)) @ wd, scaled by the routing weight,
    entirely in float32r matmuls (TF32-class precision, bf16-class speed).
  - Host combines: final[t] = contrib[expert0(t)] + contrib[expert1(t)],
    clipped to +-10.

Device blocking (per core):
  C tokens split into blocks of CB=1152.  Within a block, loop over
  I-chunks of IC=512: compute G^T/U^T [IC, CB] via PE (weights stationary,
  tokens moving), fuse silu*mul into aT chunk, then immediately run the
  down-projection for the chunk and accumulate OUT [CB, H] in SBUF.
  Weights stream HBM->SBUF once per block (2 blocks => 100MB; hidden
  under ~800us of PE work).
"""
import numpy as np
from contextlib import ExitStack

import concourse.bacc as bacc
import concourse.tile as tile
import concourse.mybir as mybir
from concourse import bass_utils


class _Exec:
    """Persistent-jit executor for the compiled Bass program over the axon
    PJRT backend (mirrors concourse.bass2jax.run_bass_via_pjrt, but keeps
    the jitted callable so repeated kernel() calls skip re-lowering).
    No donation: the kernel writes every output element it reads back."""

    def __init__(self, nc, n_cores):
        import jax
        from jax.sharding import Mesh, PartitionSpec, NamedSharding
        from jax.experimental.shard_map import shard_map
        from concourse.bass2jax import (
            _bass_exec_p, install_neuronx_cc_hook, partition_id_tensor)

        install_neuronx_cc_hook()
        self.jax = jax
        self.n_cores = n_cores
        pname = nc.partition_id_tensor.name if nc.partition_id_tensor else None
        in_names, out_names, out_avals, out_shapes = [], [], [], []
        for alloc in nc.m.functions[0].allocations:
            if not isinstance(alloc, mybir.MemoryLocationSet):
                continue
            name = alloc.memorylocations[0].name
            if alloc.kind == "ExternalInput":
                if name != pname:
                    in_names.append(name)
            elif alloc.kind == "ExternalOutput":
                shape = tuple(alloc.tensor_shape)
                dtype = mybir.dt.np(alloc.dtype)
                out_names.append(name)
                out_shapes.append((shape, dtype))
                out_avals.append(jax.core.ShapedArray(shape, dtype))
        self.in_names, self.out_names = in_names, out_names
        self.out_shapes = out_shapes
        all_in = in_names + out_names + ([pname] if pname else [])

        def _body(*args):
            operands = list(args)
            if pname is not None:
                operands.append(partition_id_tensor())
            return tuple(_bass_exec_p.bind(
                *operands,
                out_avals=tuple(out_avals),
                in_names=tuple(all_in),
                out_names=tuple(out_names),
                lowering_input_output_aliases=(),
                sim_require_finite=True,
                sim_require_nnan=True,
                nc=nc,
            ))

        devices = jax.devices()[:n_cores]
        assert len(devices) == n_cores
        mesh = Mesh(np.asarray(devices), ("core",))
        spec = PartitionSpec("core")
        self.sharding = NamedSharding(mesh, spec)
        n_args = len(in_names) + len(out_names)
        self.fn = jax.jit(
            shard_map(_body, mesh=mesh, in_specs=(spec,) * n_args,
                      out_specs=(spec,) * len(out_names), check_rep=False),
            keep_unused=True,
        )

    def run(self, in_maps):
        jax = self.jax
        concat = [
            np.concatenate(
                [np.asarray(in_maps[c][n]) for c in range(self.n_cores)],
                axis=0)
            for n in self.in_names
        ]
        concat += [
            np.zeros((self.n_cores * s[0], *s[1:]), d)
            for s, d in self.out_shapes
        ]
        dev = [jax.device_put(a, self.sharding) for a in concat]
        outs = self.fn(*dev)
        jax.block_until_ready(outs)
        per_core = []
        for c in range(self.n_cores):
            d = {}
            for i, n in enumerate(self.out_names):
                s, _ = self.out_shapes[i]
                d[n] = np.asarray(outs[i]).reshape(self.n_cores, *s)[c]
            per_core.append(d)
        return per_core

TOP_K = 2
AUX_COEF = 0.001
B, S, H, I, E = 4, 2048, 1024, 4096, 8
T = B * S
N_CORES = 8

CB = 768         # tokens per block on device
IC = 512         # I-chunk size
CSUB = 384       # moving-dim subtile for gate/up matmuls
NT = 512         # moving-dim subtile for down-proj matmuls

F32 = mybir.dt.float32
F32R = mybir.dt.float32r

_nc_cache = {}


def normalize_cx(cx):
    """Exact token capacity, adjusted so the final block decomposes into
    c-subtile widths all >= 256 (f32r matmuls drop to 1/4 rate below 256)
    and 4-aligned (ISA)."""
    cx = max(int(cx), 256)
    cx = -(-cx // 8) * 8
    r = cx % CB
    if r == 0:
        return cx
    if r < 256:
        cx += 256 - r
    elif 384 < r <= 512:
        cx += 520 - r
    return cx


def _block_plan(cx):
    """Partition cx token-columns into blocks (<= CB total each); each block
    is a list of c-subtile widths in [256, 512]."""
    k, r = divmod(cx, CB)
    blocks = [[CSUB, CSUB] for _ in range(k)]
    if r:
        if r <= 384:
            blocks.append([r])
        elif r <= 512:
            blocks.append([r])      # single wide subtile (<= 512)
        else:
            blocks.append([(r + 1) // 2, r // 2])
    return blocks


def _build_nc(Cx):
    blocks = _block_plan(Cx)          # list of width-lists; widths in [256,512]
    Cp = -(-Cx // 128) * 128          # padded row count for I/O arrays
    n_chunks = I // IC                # 8
    n_it = IC // 128                  # 4 i-tiles per chunk
    n_n = H // NT                     # 2 n-tiles
    n_kc = IC // 128                  # 4 k-tiles per chunk (down-proj)
    n_kh = H // 128                   # 8 k-tiles (gate/up contraction over H)
    n_m_max = CB // 128

    nc = bacc.Bacc("TRN2", target_bir_lowering=False, debug=False)
    xgt = nc.dram_tensor("xgt", [H, Cp], F32R, kind="ExternalInput")
    wg = nc.dram_tensor("wg", [H, I], F32R, kind="ExternalInput")
    wu = nc.dram_tensor("wu", [H, I], F32R, kind="ExternalInput")
    wd = nc.dram_tensor("wd", [I, H], F32R, kind="ExternalInput")
    scale = nc.dram_tensor("scale", [128, Cp // 128], F32, kind="ExternalInput")
    outc = nc.dram_tensor("outc", [Cp, H], F32, kind="ExternalOutput")

    xgt_r = xgt.rearrange("(ko p) c -> p ko c", p=128)     # [128, 8, Cp]
    wg_r = wg.rearrange("(ko p) i -> p ko i", p=128)       # [128, 8, I]
    wu_r = wu.rearrange("(ko p) i -> p ko i", p=128)
    wd_r = wd.rearrange("(ko p) h -> p ko h", p=128)       # [128, 32, H]
    outc_r = outc.rearrange("(mt p) h -> p mt h", p=128)   # [128, Cp/128, H]

    IH = IC // 2  # half-chunk granularity for gate/up weight DMAs

    with tile.TileContext(nc) as tc, ExitStack() as ctx:
        xg_pool = ctx.enter_context(tc.tile_pool(name="xg", bufs=4))
        wg_pool = ctx.enter_context(tc.tile_pool(name="wg", bufs=4))
        wu_pool = ctx.enter_context(tc.tile_pool(name="wu", bufs=4))
        wd_pool = ctx.enter_context(tc.tile_pool(name="wdp", bufs=2))
        a_pool = ctx.enter_context(tc.tile_pool(name="at", bufs=1))
        silu_pool = ctx.enter_context(tc.tile_pool(name="silu", bufs=3))
        out_pool = ctx.enter_context(tc.tile_pool(name="outsb", bufs=n_m_max + 3))
        sc_pool = ctx.enter_context(tc.tile_pool(name="sc", bufs=1))
        psg_pool = ctx.enter_context(tc.tile_pool(name="psg", bufs=2, space="PSUM"))
        psu_pool = ctx.enter_context(tc.tile_pool(name="psu", bufs=2, space="PSUM"))
        pso_pool = ctx.enter_context(tc.tile_pool(name="pso", bufs=4, space="PSUM"))

        sc_t = sc_pool.tile([128, Cp // 128], F32)
        nc.sync.dma_start(sc_t[:], scale[:])

        # PE warmup: dummy matmuls with no DMA dependency keep the PE busy
        # while the first real loads land, and un-throttle the HAM clock
        # gate (cold 1.2GHz -> warm 2.4GHz) before real work starts.
        wu_sb = sc_pool.tile([128, 640], F32R, tag="warm")
        nc.vector.memset(wu_sb.bitcast(F32)[:], 0.0)
        for g in range(6):
            wu_ps = pso_pool.tile([128, NT], F32, tag="ps_o")
            for k in range(8):
                nc.tensor.matmul(
                    wu_ps[:], wu_sb[:, 0:128], wu_sb[:, 128:128 + NT],
                    start=(k == 0), stop=(k == 7),
                )

        def load_gu(i0, h):
            wg_t = wg_pool.tile([128, n_kh, IH], F32R, tag="wg")
            nc.sync.dma_start(
                wg_t[:], wg_r[:, :, i0 + h * IH:i0 + (h + 1) * IH])
            wu_t = wu_pool.tile([128, n_kh, IH], F32R, tag="wu")
            nc.sync.dma_start(
                wu_t[:], wu_r[:, :, i0 + h * IH:i0 + (h + 1) * IH])
            return wg_t, wu_t

        def load_wd(ci):
            wd_t = wd_pool.tile([128, n_kc, H], F32R)
            nc.sync.dma_start(wd_t[:], wd_r[:, ci * n_kc:(ci + 1) * n_kc, :])
            return wd_t

        c0 = 0
        for b, widths in enumerate(blocks):
            bt = sum(widths)
            n_cs = len(widths)
            n_m = -(-bt // 128)
            # DMA issue order matters at block 0: first c-subtile of xgT and
            # the first gate/up half-chunk land first so PE starts ASAP.
            xg_ts = []
            xg_t = xg_pool.tile([128, n_kh, widths[0]], F32R, tag="xg")
            nc.sync.dma_start(xg_t[:], xgt_r[:, :, c0:c0 + widths[0]])
            xg_ts.append(xg_t)
            g0, u0 = load_gu(0, 0)
            s0 = widths[0]
            for cs in range(1, n_cs):
                w = widths[cs]
                xg_t = xg_pool.tile([128, n_kh, w], F32R, tag="xg")
                nc.sync.dma_start(xg_t[:], xgt_r[:, :, c0 + s0:c0 + s0 + w])
                xg_ts.append(xg_t)
                s0 += w
            g1, u1 = load_gu(0, 1)
            wd0 = load_wd(0)
            out_ts = []
            for _m in range(n_m):
                out_t = out_pool.tile([128, H], F32, tag="out")
                out_ts.append(out_t)

            for ci in range(n_chunks):
                i0 = ci * IC
                if ci == 0:
                    wg_ts, wu_ts, wd_t = [g0, g1], [u0, u1], wd0
                else:
                    wg_ts, wu_ts = [], []
                    for h in range(2):
                        wg_t, wu_t = load_gu(i0, h)
                        wg_ts.append(wg_t)
                        wu_ts.append(wu_t)
                    wd_t = load_wd(ci)

                a_t = a_pool.tile([128, n_kc, CB], F32R, tag="a_t")

                s0 = 0
                for cs in range(n_cs):
                    w = widths[cs]
                    for it in range(n_it):
                        wg_h = wg_ts[it // (n_it // 2)]
                        wu_h = wu_ts[it // (n_it // 2)]
                        ih = (it % (n_it // 2)) * 128
                        ps_g = psg_pool.tile([128, w], F32, tag="ps_g")
                        for k in range(n_kh):
                            nc.tensor.matmul(
                                ps_g[:],
                                wg_h[:, k, ih:ih + 128],
                                xg_ts[cs][:, k, :],
                                start=(k == 0), stop=(k == n_kh - 1),
                            )
                        ps_u = psu_pool.tile([128, w], F32, tag="ps_u")
                        for k in range(n_kh):
                            nc.tensor.matmul(
                                ps_u[:],
                                wu_h[:, k, ih:ih + 128],
                                xg_ts[cs][:, k, :],
                                start=(k == 0), stop=(k == n_kh - 1),
                            )
                        sl_t = silu_pool.tile([128, w], F32, tag="sl")
                        nc.scalar.activation(
                            sl_t[:], ps_g[:], mybir.ActivationFunctionType.Silu
                        )
                        nc.vector.tensor_mul(
                            a_t[:, it, s0:s0 + w], sl_t[:], ps_u[:]
                        )
                    s0 += w

                for m in range(n_m):
                    mw = min(128, bt - m * 128)
                    for n in range(n_n):
                        ps_o = pso_pool.tile([128, NT], F32, tag="ps_o")
                        for k in range(n_kc):
                            # stationary stays full 128-wide (ISA requires
                            # it); rows >= mw are garbage and never read
                            nc.tensor.matmul(
                                ps_o[:],
                                a_t[:, k, m * 128:m * 128 + 128],
                                wd_t[:, k, n * NT:(n + 1) * NT],
                                start=(k == 0), stop=(k == n_kc - 1),
                            )
                        dst = out_ts[m][0:mw, n * NT:(n + 1) * NT]
                        idx = c0 // 128 + m
                        sc_ap = sc_t[0:mw, idx:idx + 1]
                        if ci == 0:
                            nc.vector.tensor_scalar_mul(dst, ps_o[0:mw, :], sc_ap)
                        else:
                            nc.vector.scalar_tensor_tensor(
                                dst, ps_o[0:mw, :], sc_ap, dst,
                                op0=mybir.AluOpType.mult,
                                op1=mybir.AluOpType.add,
                            )

            for m in range(n_m):
                mw = min(128, bt - m * 128)
                nc.sync.dma_start(
                    outc_r[0:mw, c0 // 128 + m, :], out_ts[m][0:mw, :]
                )
            c0 += bt

    nc.compile()
    return nc


def capacity(counts):
    """Exact (normalized) device token capacity Cx."""
    return normalize_cx(int(counts.max()))


def padded(cx):
    """Row-padded capacity (I/O array sizing)."""
    return -(-cx // 128) * 128


def get_nc(C):
    if C not in _nc_cache:
        _nc_cache[C] = _build_nc(C)
    return _nc_cache[C]


_exec_cache = {}


def get_exec(C):
    if C not in _exec_cache:
        _exec_cache[C] = _Exec(get_nc(C), N_CORES)
    return _exec_cache[C]


def _route(hidden_states, router_w):
    """Host router: returns per-core inputs metadata + losses (fp32)."""
    x = np.clip(np.asarray(hidden_states, np.float32), -10.0, 10.0)
    xf = x.reshape(T, H)
    logits = xf @ np.asarray(router_w, np.float32)           # [T, E]
    mx = logits.max(axis=-1, keepdims=True)
    ex = np.exp(logits - mx)
    sex = ex.sum(axis=-1, keepdims=True)
    probs = ex / sex
    lse = (mx + np.log(sex)).ravel()
    z_loss = np.float32(np.mean(lse ** 2))

    i1 = probs.argmax(axis=-1)
    p1 = probs[np.arange(T), i1]
    pm = probs.copy()
    pm[np.arange(T), i1] = -np.inf
    i2 = pm.argmax(axis=-1)
    p2 = probs[np.arange(T), i2]
    denom = p1 + p2
    w1 = p1 / denom
    w2 = p2 / denom

    counts = np.zeros(E, np.int64)
    np.add.at(counts, i1, 1)
    np.add.at(counts, i2, 1)
    frac = counts.astype(np.float32) / T / TOP_K
    lb_loss = np.float32(AUX_COEF * E * np.sum(frac * probs.mean(axis=0)))
    return xf, probs, i1, i2, w1, w2, counts, lb_loss, z_loss


def build_src(i1, i2, C):
    """Host combine indices: token t's two contributions live at rows
    src0[t], src1[t] of the concatenated [E*C, H] device output."""
    src0 = np.empty(T, np.int64)
    src1 = np.empty(T, np.int64)
    for e in range(E):
        idx = np.flatnonzero((i1 == e) | (i2 == e))
        first = i1[idx] == e
        rank = np.arange(len(idx))
        src0[idx[first]] = e * C + rank[first]
        src1[idx[~first]] = e * C + rank[~first]
    return src0, src1


def build_in_maps(inputs, xf, i1, i2, w1, w2, C):
    w_gate = np.ascontiguousarray(np.asarray(inputs["w_gate"], np.float32))
    w_up = np.ascontiguousarray(np.asarray(inputs["w_up"], np.float32))
    w_down = np.ascontiguousarray(np.asarray(inputs["w_down"], np.float32))
    in_maps = []
    for e in range(E):
        idx = np.flatnonzero((i1 == e) | (i2 == e))
        ne = len(idx)
        first = i1[idx] == e
        xg = np.zeros((C, H), np.float32)
        xg[:ne] = xf[idx]
        sc = np.zeros(C, np.float32)
        sc[:ne] = np.where(first, w1[idx], w2[idx]).astype(np.float32)
        in_maps.append({
            "xgt": np.ascontiguousarray(xg.T),
            "wg": w_gate[e],
            "wu": w_up[e],
            "wd": w_down[e],
            "scale": np.ascontiguousarray(sc.reshape(C // 128, 128).T),
        })
    return in_maps


def kernel(**inputs):
    hidden_states = np.asarray(inputs["hidden_states"], np.float32)
    router_w = np.asarray(inputs["router_w"], np.float32)

    xf, probs, i1, i2, w1, w2, counts, lb_loss, z_loss = _route(
        hidden_states, router_w
    )

    Cx = capacity(counts)
    Cp = padded(Cx)
    in_maps = build_in_maps(inputs, xf, i1, i2, w1, w2, Cp)
    src0, src1 = build_src(i1, i2, Cp)

    outc_all = _run_device(Cx, in_maps)
    if outc_all is None:
        # Emergency fallback: exact numpy compute (slow but correct).
        outc_all = _numpy_experts(in_maps, Cp)

    out = outc_all[src0] + outc_all[src1]
    out = np.clip(out, -10.0, 10.0).reshape(B, S, H)
    return out, lb_loss, z_loss


def _run_device(Cx, in_maps):
    import sys
    import traceback
    try:
        results = get_exec(Cx).run(in_maps)
        return np.concatenate(
            [results[c]["outc"] for c in range(N_CORES)], axis=0)
    except BaseException as e:
        if isinstance(e, (KeyboardInterrupt, SystemExit)):
            raise
        traceback.print_exc()
        print("kernel: persistent-jit path failed, trying spmd runner",
              file=sys.stderr, flush=True)
    try:
        res = bass_utils.run_bass_kernel_spmd(
            get_nc(Cx), in_maps, core_ids=list(range(N_CORES)))
        return np.concatenate(
            [res.results[c]["outc"] for c in range(N_CORES)], axis=0)
    except BaseException as e:
        if isinstance(e, (KeyboardInterrupt, SystemExit)):
            raise
        traceback.print_exc()
        print("kernel: device paths failed, falling back to numpy",
              file=sys.stderr, flush=True)
    return None


def _numpy_experts(in_maps, Cp):
    outs = []
    for m in in_maps:
        x = m["xgt"].T
        g = x @ m["wg"]
        u = x @ m["wu"]
        a = (g / (1.0 + np.exp(-g))) * u
        o = (a @ m["wd"]) * m["scale"].T.reshape(-1, 1)
        outs.append(o.astype(np.float32))
    return np.concatenate(outs, axis=0)
